# revision 33
# baseline (speedup 1.0000x reference)
"""TRN2 Bass kernel for CausalSCMLayer: z_causal = z @ (I - tril(A_raw,-1))^{-1}.

Math: A = tril(A_raw, -1) is strictly lower triangular (nilpotent), so
W = (I - A)^{-1} is unit lower triangular, tiny (256x256), and shared
across the whole batch. It is computed exactly on the host (float64
inverse) and passed in as an input; the device does only the batched
streaming work out = z @ W.

The batched matmul runs on the PE in float32r (TF32-like, ~11-bit
mantissa, exact products, fp32 accumulate): each 128-row tile of z is
transposed on the PE (fp32, exact), rounded to f32r by the ACT engine's
PSUM->SBUF round-copy, multiplied by the f32r W, and the fp32 PSUM result
is copied out by the DVE. End-to-end error ~1e-4, dominated by the tf32
rounding of z itself.

DMA: z/out move in variable-size supertiles rearranged "(p n) v" so each
SBUF partition line is one contiguous >=8 KiB HBM span (128 descriptors
per DMA). Per-queue throughput is flat at ~26.5 B/ns for descriptors
>=4 KiB (16 queues x 26.5 ~= the per-core HBM share), and every DMA
instruction adds a ~0.3-0.6 us completion overhead that serializes on
one unlucky queue, so the instruction count is kept minimal: 1 MiB
starter/closer supertiles for a fast ramp and a short final drain, 2 MiB
supertiles in the middle; stores split in halves only for the first
supertile (launches the store stream early) and the last (so the final
post-compute drain is ~0.5 MiB).

Sharding: data-parallel over the batch axis across 8 cores; W replicated.
"""

import numpy as np

import concourse.bass as bass
import concourse.tile as tile
from concourse import bacc, mybir
from concourse.bass_utils import run_bass_kernel_spmd
from concourse.masks import make_identity

F32 = mybir.dt.float32
F32R = mybir.dt.float32r

N_CORES = 8
BATCH = 131072
NVARS = 256
BC = BATCH // N_CORES          # rows per core
# supertile sizes in 128-row tiles (sum = BC/128 = 128 tiles)
SUPER_TILES = [8, 16, 16, 16, 16, 16, 16, 16, 8]
assert sum(SUPER_TILES) * 128 == BC

_CACHE = {}


def _build_nc():
    nc = bacc.Bacc("TRN2", target_bir_lowering=False, debug=False,
                   num_devices=N_CORES)
    z = nc.dram_tensor("z", [BC, NVARS], F32, kind="ExternalInput").ap()
    r = nc.dram_tensor("r", [128, 2 * NVARS], F32, kind="ExternalInput").ap()
    out = nc.dram_tensor("out", [BC, NVARS], F32, kind="ExternalOutput").ap()

    # (p n) within a supertile: partition p holds rows p*T .. p*T+T-1 —
    # one contiguous T KiB HBM chunk per partition. Row<->partition
    # mapping is identical on load and store and every batch row is
    # independent, so compute tiles n are just a consistent 128-row
    # subset.
    def z_sup(row0, T):
        return z[row0:row0 + T * 128, :].rearrange(
            "(p n) v -> p n v", p=128, n=T)

    def o_sup(row0, T):
        return out[row0:row0 + T * 128, :].rearrange(
            "(p n) v -> p n v", p=128, n=T)

    with tile.TileContext(nc) as tc:
        with (
            tc.tile_pool(name="const", bufs=1) as cp,
            tc.tile_pool(name="zin8", bufs=2) as zin8_pool,
            tc.tile_pool(name="zin16", bufs=5) as zin16_pool,
            tc.tile_pool(name="outb8", bufs=2) as outb8_pool,
            tc.tile_pool(name="outb16", bufs=4) as outb16_pool,
            tc.tile_pool(name="ztr", bufs=16) as ztr_pool,
            tc.tile_pool(name="psT", bufs=3, space="PSUM") as psT_pool,
            tc.tile_pool(name="psC", bufs=5, space="PSUM") as psC_pool,
        ):
            ident = cp.tile([128, 128], F32)
            Wm = cp.tile([128, 2 * NVARS], F32)
            Wmr = cp.tile([128, 2 * NVARS], F32R)
            # W on the ACT HWDGE ring so the z loads own the SP ring from
            # the first cycle; it lands well before the first matmul.
            nc.scalar.dma_start(Wm[:], r)
            make_identity(nc, ident[:])
            # DVE round-copy to f32r (the PE's fp32r path requires
            # pre-rounded operands).
            nc.vector.tensor_copy(Wmr[:], Wm[:])
            Wmr0 = Wmr[:, 0:256]    # rows 0:128 of W
            Wmr1 = Wmr[:, 256:512]  # rows 128:256 of W

            # PE p-state warm-up: HAM starts the PE clock-gated at 1.2 GHz
            # and only un-throttles after ~3.4us of sustained activity.
            # Burn the load-wait window with dep-free transposes so real
            # work runs at 2.4 GHz.
            warm = psT_pool.tile([128, 256], F32, tag="pT", name="warmps")
            for _ in range(10):
                nc.tensor.transpose(warm[:, 0:128], ident[:], ident[:])

            # main loop: out = z @ W, 128-row tiles, software-pipelined
            # by SKEW tiles so the PE never stalls on the ACT round-copy.
            n_super = len(SUPER_TILES)
            zin_t = {}
            outb_t = {}
            work = []
            row0 = 0
            row0s = []
            # Alternate early loads between the SP and ACT HWDGE rings:
            # descriptor prep for the first loads then runs on two DGEs in
            # parallel, so all 16 DMA queues are fed ~2us sooner. Only
            # wait-free loads (before any zin buffer recycling) may leave
            # the SP ring: a WAR-waiting load at the head of the ACT ring
            # would deadlock against the zr copies queued behind it.
            load_ring = {1: nc.scalar, 3: nc.scalar, 5: nc.scalar}
            # wait-free loads: s0/s8 (zin8, 2 bufs), s1-s5 (zin16, 5 bufs);
            # s6, s7 recycle zin16 bufs and must stay on the SP ring.
            for s, T in enumerate(SUPER_TILES):
                row0s.append(row0)
                zp = zin8_pool if T == 8 else zin16_pool
                op = outb8_pool if T == 8 else outb16_pool
                zin_t[s] = zp.tile([128, T, 256], F32,
                                   tag=f"zin{T}", name=f"zin{s}")
                load_ring.get(s, nc.sync).dma_start(zin_t[s][:],
                                                    z_sup(row0, T))
                outb_t[s] = op.tile([128, T, 256], F32,
                                    tag=f"outb{T}", name=f"outb{s}")
                for n in range(T):
                    work.append((s, n))
                row0 += T * 128

            from collections import deque
            SKEW = 4  # transposes run 4 tiles ahead of the matmuls
            pending = deque()
            done_in_super = {s: 0 for s in range(n_super)}

            def flush(p):
                zr, out_ap, s = p
                pC = psC_pool.tile([128, 256], F32, tag="pC", name=f"pC{s}")
                nc.tensor.matmul(pC[:], zr[:, 0:128], Wmr0,
                                 start=True, stop=False)
                nc.tensor.matmul(pC[:], zr[:, 128:256], Wmr1,
                                 start=False, stop=True)
                nc.vector.tensor_copy(out_ap, pC[:])
                done_in_super[s] += 1
                T = SUPER_TILES[s]
                osup = o_sup(row0s[s], T)
                # last supertile stores in halves so the final
                # post-compute drain is only ~0.5 MiB; others store whole
                # (the queues are load-saturated early regardless).
                if s == n_super - 1:
                    cuts = [T // 2, T]
                else:
                    cuts = [T]
                d = done_in_super[s]
                if d in cuts:
                    lo = 0 if d == cuts[0] else cuts[cuts.index(d) - 1]
                    nc.gpsimd.dma_start(osup[:, lo:d, :],
                                        outb_t[s][:, lo:d, :])

            for ti, (s, n) in enumerate(work):
                zt = zin_t[s][:, n, :]
                pT = psT_pool.tile([128, 256], F32, tag="pT", name=f"pT{s}_{n}")
                nc.tensor.transpose(pT[:, 0:128], zt[:, 0:128], ident[:])
                nc.tensor.transpose(pT[:, 128:256], zt[:, 128:256], ident[:])
                zr = ztr_pool.tile([128, 256], F32R, tag="zr", name=f"zr{s}_{n}")
                nc.scalar.copy(zr[:], pT[:])
                pending.append((zr, outb_t[s][:, n, :], s))
                if len(pending) > SKEW:
                    flush(pending.popleft())
            while pending:
                flush(pending.popleft())

    nc.compile()
    return nc


def _get_nc():
    if "nc" not in _CACHE:
        _CACHE["nc"] = _build_nc()
    return _CACHE["nc"]


def kernel(z_exogenous, A_raw):
    # NTFF tracing needs antenv.axon_hooks; if BASS_TRACE is set in an
    # environment that lacks it, run_bass_kernel_spmd would crash.
    import os
    try:
        import antenv.axon_hooks  # noqa: F401
    except ImportError:
        os.environ["BASS_NEVER_TRACE"] = "1"

    z = np.ascontiguousarray(np.asarray(z_exogenous, dtype=np.float32))
    A = np.ascontiguousarray(np.asarray(A_raw, dtype=np.float32))
    assert z.shape == (BATCH, NVARS) and A.shape == (NVARS, NVARS)

    # W = (I - A)^{-1}, computed exactly in float64 on the host
    # (256x256, ~microseconds) and packed as [rows 0:128 | rows 128:256].
    A64 = np.tril(A.astype(np.float64), -1)
    eye = np.eye(NVARS, dtype=np.float64)
    W = np.linalg.inv(eye - A64).astype(np.float32)
    Wm = np.ascontiguousarray(
        np.concatenate([W[0:128, :], W[128:256, :]], axis=1))

    nc = _get_nc()
    in_maps = [
        {"z": z[i * BC:(i + 1) * BC], "r": Wm} for i in range(N_CORES)
    ]
    res = run_bass_kernel_spmd(nc, in_maps, core_ids=list(range(N_CORES)))
    kernel.last_exec_time_ns = res.exec_time_ns
    kernel.last_results = res
    return np.concatenate([res.results[i]["out"] for i in range(N_CORES)], axis=0)


# revision 35
# speedup vs baseline: 1.1208x; 1.1208x over previous
"""TRN2 Bass kernel for CausalSCMLayer: z_causal = z @ (I - tril(A_raw,-1))^{-1}.

Math: A = tril(A_raw, -1) is strictly lower triangular (nilpotent), so
W = (I - A)^{-1} is unit lower triangular, tiny (256x256), and shared
across the whole batch. It is computed exactly on the host (float64
inverse) and passed in as an input; the device does only the batched
streaming work out = z @ W.

The batched matmul runs on the PE in float32r (TF32-like, ~11-bit
mantissa, exact products, fp32 accumulate): each 128-row tile of z is
transposed on the PE (fp32, exact), rounded to f32r by the ACT engine's
PSUM->SBUF round-copy, multiplied by the f32r W, and the fp32 PSUM result
is copied out by the DVE. End-to-end error ~1e-4, dominated by the tf32
rounding of z itself.

DMA: z/out move in variable-size supertiles rearranged "(p n) v" so each
SBUF partition line is one contiguous >=8 KiB HBM span (128 descriptors
per DMA). Per-queue throughput is flat at ~26.5 B/ns for descriptors
>=4 KiB (16 queues x 26.5 ~= the per-core HBM share), and every DMA
instruction adds a ~0.3-0.6 us completion overhead that serializes on
one unlucky queue, so the instruction count is kept minimal: 1 MiB
starter/closer supertiles for a fast ramp and a short final drain, 2 MiB
supertiles in the middle; stores split in halves only for the first
supertile (launches the store stream early) and the last (so the final
post-compute drain is ~0.5 MiB).

Sharding: data-parallel over the batch axis across 8 cores; W replicated.
"""

import numpy as np

import concourse.bass as bass
import concourse.tile as tile
from concourse import bacc, mybir
from concourse.bass_utils import run_bass_kernel_spmd
from concourse.masks import make_identity

F32 = mybir.dt.float32
F32R = mybir.dt.float32r

N_CORES = 8
BATCH = 131072
NVARS = 256
BC = BATCH // N_CORES          # rows per core
# supertile sizes in 128-row tiles (sum = BC/128 = 128 tiles)
SUPER_TILES = [8, 8, 16, 16, 16, 16, 16, 16, 8, 8]
assert sum(SUPER_TILES) * 128 == BC

_CACHE = {}


def _build_nc():
    nc = bacc.Bacc("TRN2", target_bir_lowering=False, debug=False,
                   num_devices=N_CORES)
    z = nc.dram_tensor("z", [BC, NVARS], F32, kind="ExternalInput").ap()
    r = nc.dram_tensor("r", [128, 2 * NVARS], F32, kind="ExternalInput").ap()
    out = nc.dram_tensor("out", [BC, NVARS], F32, kind="ExternalOutput").ap()

    # (p n) within a supertile: partition p holds rows p*T .. p*T+T-1 —
    # one contiguous T KiB HBM chunk per partition. Row<->partition
    # mapping is identical on load and store and every batch row is
    # independent, so compute tiles n are just a consistent 128-row
    # subset.
    def z_sup(row0, T):
        return z[row0:row0 + T * 128, :].rearrange(
            "(p n) v -> p n v", p=128, n=T)

    def o_sup(row0, T):
        return out[row0:row0 + T * 128, :].rearrange(
            "(p n) v -> p n v", p=128, n=T)

    with tile.TileContext(nc) as tc:
        with (
            tc.tile_pool(name="const", bufs=1) as cp,
            tc.tile_pool(name="zin8", bufs=2) as zin8_pool,
            tc.tile_pool(name="zin16", bufs=4) as zin16_pool,
            tc.tile_pool(name="outb8", bufs=2) as outb8_pool,
            tc.tile_pool(name="outb16", bufs=4) as outb16_pool,
            tc.tile_pool(name="ztr", bufs=16) as ztr_pool,
            tc.tile_pool(name="psT", bufs=3, space="PSUM") as psT_pool,
            tc.tile_pool(name="psC", bufs=5, space="PSUM") as psC_pool,
        ):
            ident = cp.tile([128, 128], F32)
            Wm = cp.tile([128, 2 * NVARS], F32)
            Wmr = cp.tile([128, 2 * NVARS], F32R)
            # W on the ACT HWDGE ring so the z loads own the SP ring from
            # the first cycle; it lands well before the first matmul.
            nc.scalar.dma_start(Wm[:], r)
            make_identity(nc, ident[:])
            # DVE round-copy to f32r (the PE's fp32r path requires
            # pre-rounded operands).
            nc.vector.tensor_copy(Wmr[:], Wm[:])
            Wmr0 = Wmr[:, 0:256]    # rows 0:128 of W
            Wmr1 = Wmr[:, 256:512]  # rows 128:256 of W

            # PE p-state warm-up: HAM starts the PE clock-gated at 1.2 GHz
            # and only un-throttles after ~3.4us of sustained activity.
            # Burn the load-wait window with dep-free transposes so real
            # work runs at 2.4 GHz.
            warm = psT_pool.tile([128, 256], F32, tag="pT", name="warmps")
            for _ in range(10):
                nc.tensor.transpose(warm[:, 0:128], ident[:], ident[:])

            # main loop: out = z @ W, 128-row tiles, software-pipelined
            # by SKEW tiles so the PE never stalls on the ACT round-copy.
            n_super = len(SUPER_TILES)
            zin_t = {}
            outb_t = {}
            work = []
            row0 = 0
            row0s = []
            # Alternate early loads between the SP and ACT HWDGE rings:
            # descriptor prep for the first loads then runs on two DGEs in
            # parallel, so all 16 DMA queues are fed ~2us sooner. Only
            # wait-free loads (before any zin buffer recycling) may leave
            # the SP ring: a WAR-waiting load at the head of the ACT ring
            # would deadlock against the zr copies queued behind it.
            load_ring = {1: nc.scalar, 3: nc.scalar, 5: nc.scalar}
            # wait-free loads: s0/s1 (zin8, 2 bufs), s2-s5 (zin16, 4
            # bufs); s6-s9 recycle buffers and must stay on the SP ring.
            for s, T in enumerate(SUPER_TILES):
                row0s.append(row0)
                zp = zin8_pool if T == 8 else zin16_pool
                op = outb8_pool if T == 8 else outb16_pool
                zin_t[s] = zp.tile([128, T, 256], F32,
                                   tag=f"zin{T}", name=f"zin{s}")
                load_ring.get(s, nc.sync).dma_start(zin_t[s][:],
                                                    z_sup(row0, T))
                outb_t[s] = op.tile([128, T, 256], F32,
                                    tag=f"outb{T}", name=f"outb{s}")
                for n in range(T):
                    work.append((s, n))
                row0 += T * 128

            from collections import deque
            SKEW = 4  # transposes run 4 tiles ahead of the matmuls
            pending = deque()
            done_in_super = {s: 0 for s in range(n_super)}

            def flush(p):
                zr, out_ap, s = p
                pC = psC_pool.tile([128, 256], F32, tag="pC", name=f"pC{s}")
                nc.tensor.matmul(pC[:], zr[:, 0:128], Wmr0,
                                 start=True, stop=False)
                nc.tensor.matmul(pC[:], zr[:, 128:256], Wmr1,
                                 start=False, stop=True)
                nc.vector.tensor_copy(out_ap, pC[:])
                done_in_super[s] += 1
                T = SUPER_TILES[s]
                osup = o_sup(row0s[s], T)
                # first and last supertiles store in halves: the first
                # launches the store stream early, the last keeps the
                # final post-compute drain to ~0.5 MiB; middles store whole.
                if s == 0 or s == n_super - 1:
                    cuts = [T // 2, T]
                else:
                    cuts = [T]
                d = done_in_super[s]
                if d in cuts:
                    lo = 0 if d == cuts[0] else cuts[cuts.index(d) - 1]
                    nc.gpsimd.dma_start(osup[:, lo:d, :],
                                        outb_t[s][:, lo:d, :])

            for ti, (s, n) in enumerate(work):
                zt = zin_t[s][:, n, :]
                pT = psT_pool.tile([128, 256], F32, tag="pT", name=f"pT{s}_{n}")
                nc.tensor.transpose(pT[:, 0:128], zt[:, 0:128], ident[:])
                nc.tensor.transpose(pT[:, 128:256], zt[:, 128:256], ident[:])
                zr = ztr_pool.tile([128, 256], F32R, tag="zr", name=f"zr{s}_{n}")
                nc.scalar.copy(zr[:], pT[:])
                pending.append((zr, outb_t[s][:, n, :], s))
                if len(pending) > SKEW:
                    flush(pending.popleft())
            while pending:
                flush(pending.popleft())

    nc.compile()
    return nc


def _get_nc():
    if "nc" not in _CACHE:
        _CACHE["nc"] = _build_nc()
    return _CACHE["nc"]


def kernel(z_exogenous, A_raw):
    # NTFF tracing needs antenv.axon_hooks; if BASS_TRACE is set in an
    # environment that lacks it, run_bass_kernel_spmd would crash.
    import os
    try:
        import antenv.axon_hooks  # noqa: F401
    except ImportError:
        os.environ["BASS_NEVER_TRACE"] = "1"

    z = np.ascontiguousarray(np.asarray(z_exogenous, dtype=np.float32))
    A = np.ascontiguousarray(np.asarray(A_raw, dtype=np.float32))
    assert z.shape == (BATCH, NVARS) and A.shape == (NVARS, NVARS)

    # W = (I - A)^{-1}, computed exactly in float64 on the host
    # (256x256, ~microseconds) and packed as [rows 0:128 | rows 128:256].
    A64 = np.tril(A.astype(np.float64), -1)
    eye = np.eye(NVARS, dtype=np.float64)
    W = np.linalg.inv(eye - A64).astype(np.float32)
    Wm = np.ascontiguousarray(
        np.concatenate([W[0:128, :], W[128:256, :]], axis=1))

    nc = _get_nc()
    in_maps = [
        {"z": z[i * BC:(i + 1) * BC], "r": Wm} for i in range(N_CORES)
    ]
    res = run_bass_kernel_spmd(nc, in_maps, core_ids=list(range(N_CORES)))
    kernel.last_exec_time_ns = res.exec_time_ns
    kernel.last_results = res
    return np.concatenate([res.results[i]["out"] for i in range(N_CORES)], axis=0)
